# revision 1
# baseline (speedup 1.0000x reference)
"""Multi-head attention (Vaswani) on Trainium2, head-parallel across 8 NeuronCores.

Problem shapes (hardcoded):
  h:   [B=2, G=2048, D=128] f32
  W_Q/W_K/W_V: [H=8, D=128, K=16] f32
  out: [B=2, H=8, G=2048, V=16] f32  = softmax(0.25 * (h@Wq) @ (h@Wk)^T) @ (h@Wv)

Sharding: one head per core (8 heads / 8 cores). Each core receives the full h
plus its head's weight slices, computes [B, G, V]; host stacks on the head axis.

Per-core plan (v4). The baseline (v2, kernel_v2_baseline.py, 68452 ns) was
lock-step saturated on BOTH the Scalar engine (exp stream, ~64us busy) and
PE (compat+AV fp32r streams + staging, ~64us). v4 attacks both:

  1. exp offload: a Schraudolph fast-exp (one DVE tensor_scalar mult-add
     writing int16 bits that ARE the bf16 exp value) handles `n_dve` of the
     64 (batch, q-slice, chunk-pair) units; ACT does true exp for the rest.
     Softmax cancels the systematic exp error; measured end-to-end l2 with
     ALL units on DVE is 6.8e-3, so n_dve~14 adds ~3e-3.
  2. compat row-packing: chunk pairs run CONCURRENTLY in PE row groups 0/32
     (tile_position), halving the compat stream. Needs kT/qT replicated at
     partitions 0-15 and 32-47, which the projection matmuls produce for
     free via replicated weight columns (wq2/wk2 [128,64], zero-padded).
  3. bf16 matmul operand tiles (hT/qT2/kT2/vp/at): same PE cycles, but FWL
     weight loads, cheaper staging copies, and no f32r "produced rounded"
     constraint on the bitcast exp trick. Accuracy stack end to end:
     l2 = 8.25e-3 on HW vs the 2e-2 gate.
  4. DVE diet (DVE is the exp-offload budget): 4 transposes / 4 v-proj
     chunks funneled through ONE psum bank each -> single wide copies;
     normalize uses one [128,68] transpose target, one strided reciprocal
     [128,4] and one 0-stride-broadcast tensor_tensor multiply per slice.

Per (batch, 512-wide q-slice): 8 chunk-pair units: packed compat MMs into a
[128,1024] PSUM tile (2 banks), one 1024-wide exp (ACT or DVE), two AV MMs
accumulating oT[17,512] (ones column in v' accumulates the softmax
denominator). Slice ends: oT -> sbuf, 4 PE transposes into one [128,68]
psum tile, reciprocal+broadcast-mult, one out-DMA per slice. Input staging
for the next batch is popped a few ops per unit into the main loop
(emission order is dependency order for Tile — see the npre comment).

cfg fp8_av=True switches ACT units to fp8e4 attn/v' with paired DoubleRow
AV matmuls: correct on HW (l2 1.54e-2, exp bias -3.5 to dodge e4m3
overflow->NaN) but measured SLOWER than the bf16 path — off by default.

Interleaved A/B slope bench under identical contention (compare.py):
v4 ~107 us/rep vs v2 ~177 us/rep -> ~1.65x faster.
"""

import numpy as np

B, G, D = 2, 2048, 128
H, K, V = 8, 16, 16
N_CORES = 8
P = 128
GT = G // P          # 16 key chunks of 128
QB = 512             # q-slice width (one fp32 PSUM bank)
NSL = G // QB        # 4 q-slices per batch
NPAIR = GT // 2      # 8 chunk pairs per slice
VP1 = V + 1          # v' width (ones column appended)
VPW = VP1            # v' chunk stride

DEFAULT_CFG = {
    "n_dve": 18,       # of the 64 units, how many exp on DVE (Schraudolph)
    "fp8_av": False,   # ACT units: fp8 attn + v', paired DoubleRow AV MMs
    "at_bufs": 8,      # attnT sbuf buffers (deeper exp->AV look-ahead)
    "pc_bufs": 2,      # compat psum buffers
    "reps": 1,         # repeat whole kernel body (for HW slope timing)
    "pops": 3,         # staged ops popped per unit
}

_CACHE = {}


def _build(cfg_key):
    cfg = dict(DEFAULT_CFG)
    cfg.update(dict(cfg_key))
    import concourse.bacc as bacc
    import concourse.mybir as mybir
    from concourse.tile import TileContext
    from concourse.masks import make_identity

    f32 = mybir.dt.float32
    bf16 = mybir.dt.bfloat16
    fp8 = mybir.dt.float8e4
    i16 = mybir.dt.int16
    EXP = mybir.ActivationFunctionType.Exp
    MULT = mybir.AluOpType.mult
    ADD = mybir.AluOpType.add
    DR = mybir.MatmulPerfMode.DoubleRow
    fp8_av = bool(cfg["fp8_av"])
    # With fp8 attn weights the exp must be shifted down so e^z fits e4m3's
    # [2^-9, 448] range: z = 0.25*x - 2 (logits are within +-8.5; the shift
    # cancels between softmax numerator and denominator).
    EBIAS = -3.5 if fp8_av else 0.0
    # bf16-space Schraudolph constants (exp(0.25*x + EBIAS) via bit trick):
    # bits16 = rint((0.25*x + EBIAS) * 2^7/ln2 + 127*2^7), as bf16.
    A16 = float(0.25 * (2 ** 7) / np.log(2.0))
    B16 = float(127 * 2 ** 7 + EBIAS * (2 ** 7) / np.log(2.0))

    n_dve = int(cfg["n_dve"])
    NU = B * NSL * NPAIR  # 64 units per rep

    def unit_on_dve(uid):
        return (uid + 1) * n_dve // NU > uid * n_dve // NU

    nc = bacc.Bacc("TRN2", debug=False, enable_asserts=False,
                   target_bir_lowering=False)
    h_d = nc.dram_tensor("h", [B, G, D], f32, kind="ExternalInput").ap()
    wq_d = nc.dram_tensor("wq", [D, K], f32, kind="ExternalInput").ap()
    wk_d = nc.dram_tensor("wk", [D, K], f32, kind="ExternalInput").ap()
    wv_d = nc.dram_tensor("wv", [D, V], f32, kind="ExternalInput").ap()
    out_d = nc.dram_tensor("out", [B, G, V], f32, kind="ExternalOutput").ap()

    with TileContext(nc) as tc:
        with tc.tile_pool(name="const", bufs=1) as cpool, \
             tc.tile_pool(name="sc", bufs=2, space="PSUM") as scpool, \
             tc.tile_pool(name="pc", bufs=cfg["pc_bufs"],
                          space="PSUM") as pcpool, \
             tc.tile_pool(name="po", bufs=2, space="PSUM") as popool, \
             tc.tile_pool(name="att", bufs=cfg["at_bufs"]) as apool:
            ident = cpool.tile([P, P], f32)
            make_identity(nc, ident)
            warm = cpool.tile([P, 1], f32)
            nc.scalar.activation(warm, ident[:, 0:1], EXP)
            biast = cpool.tile([P, 1], f32)
            nc.vector.memset(biast, EBIAS)
            w_sb = cpool.tile([D, 3 * K], f32)
            wq2 = cpool.tile([D, 64], bf16)
            wk2 = cpool.tile([D, 64], bf16)
            wv_r = cpool.tile([D, V], bf16)

            def load_w():
                nc.sync.dma_start(w_sb[:, 0:K], wq_d)
                nc.sync.dma_start(w_sb[:, K:2 * K], wk_d)
                nc.sync.dma_start(w_sb[:, 2 * K:3 * K], wv_d)

            def build_w():
                nc.vector.memset(wq2, 0.0)
                nc.vector.memset(wk2, 0.0)
                for g in (0, 32):
                    nc.vector.tensor_copy(wq2[:, g:g + K], w_sb[:, 0:K])
                    nc.vector.tensor_copy(wk2[:, g:g + K], w_sb[:, K:2 * K])
                nc.vector.tensor_copy(wv_r, w_sb[:, 2 * K:3 * K])

            hA_b, hT_b, qT_b, kT_b, vp_b, v8_b, ob_b = [], [], [], [], [], [], []
            for b in range(B):
                hA_b.append(cpool.tile([P, G], f32, name=f"hA{b}"))
                hT_b.append(cpool.tile([P, G], bf16, name=f"hT{b}"))
                qT_b.append(cpool.tile([48, G], bf16, name=f"qT{b}"))
                kT_b.append(cpool.tile([48, G], bf16, name=f"kT{b}"))
                vp_b.append(cpool.tile([P, GT * VPW], bf16, name=f"vp{b}"))
                if fp8_av:
                    # v' chunk pairs for DoubleRow: pair p at 64p, chunks at
                    # +0 and +32 (Ko step 32 B), ones column at +16/+48.
                    v8_b.append(cpool.tile([P, NPAIR * 64], fp8,
                                           name=f"v8{b}"))
                else:
                    v8_b.append(None)
                ob_b.append(cpool.tile([P, GT * V], f32, name=f"ob{b}"))

            def init_vp():
                for b in range(B):
                    nc.vector.memset(
                        vp_b[b].rearrange("p (t w) -> p t w", w=VPW)[:, :, V:],
                        1.0)
                    if fp8_av:
                        nc.vector.memset(
                            v8_b[b].rearrange(
                                "p (x w) -> p x w", w=32)[:, :, V:V + 1],
                            1.0)

            def phase1_ops(b):
                """Input staging for batch b, in dependency order; popped a
                few per unit inside the previous batch's main loop."""
                hA, hT, qT2, kT2, vp, vp8 = (hA_b[b], hT_b[b], qT_b[b],
                                             kT_b[b], vp_b[b], v8_b[b])

                def dmaq(qq):
                    nc.sync.dma_start(
                        hA[:, qq * 4 * P:(qq + 1) * 4 * P].rearrange(
                            "p (t d) -> p t d", t=4),
                        h_d[b, qq * 4 * P:(qq + 1) * 4 * P, :].rearrange(
                            "(t p) d -> p t d", p=P))

                def trq(qq):
                    # 4 chunk transposes into one psum bank, single copy out
                    # (f32r-bitcast transposes rejected by walrus codegen)
                    pt = scpool.tile([P, QB], f32, tag="s", name="pt")
                    for j in range(4):
                        t = 4 * qq + j
                        nc.tensor.transpose(pt[:, j * P:(j + 1) * P],
                                            hA[:, t * P:(t + 1) * P], ident)
                    nc.vector.tensor_copy(
                        hT[:, qq * 4 * P:(qq + 1) * 4 * P], pt)

                def proj(qb, w2, dst):
                    sl = slice(qb * QB, (qb + 1) * QB)
                    pq = scpool.tile([P, QB], f32, tag="s", name="pq")
                    nc.tensor.matmul(pq[0:64, :], w2, hT[:, sl],
                                     start=True, stop=True)
                    nc.vector.tensor_copy(dst[0:48, sl], pq[0:48, :])

                def vprojq(qq):
                    # 4 chunks' v' into one psum tile, then one strided copy
                    pvv = scpool.tile([P, QB], f32, tag="s", name="pvv")
                    for j in range(4):
                        t = 4 * qq + j
                        nc.tensor.matmul(pvv[:, j * V:(j + 1) * V],
                                         hT[:, t * P:(t + 1) * P],
                                         wv_r, start=True, stop=True)
                    src = pvv[:, 0:4 * V].rearrange("p (j v) -> p j v", v=V)
                    nc.vector.tensor_copy(
                        vp.rearrange("p (t w) -> p t w", w=VPW)
                        [:, 4 * qq:4 * qq + 4, 0:V], src)
                    if fp8_av:
                        nc.vector.tensor_copy(
                            v8.rearrange("p (x w) -> p x w", w=32)
                            [:, 4 * qq:4 * qq + 4, 0:V], src)

                v8 = vp8
                ops = [lambda qq=qq: dmaq(qq) for qq in range(4)]
                for qq in range(4):
                    ops.append(lambda qq=qq: trq(qq))
                    ops.append(lambda qq=qq: proj(qq, wk2, kT2))
                    # projq before vprojq: projq gates the quarter's first
                    # compat MMs, vprojq only the AV ~1us later
                    ops.append(lambda qq=qq: proj(qq, wq2, qT2))
                    ops.append(lambda qq=qq: vprojq(qq))
                return ops

            units = [(rr, bb) for rr in range(cfg["reps"])
                     for bb in range(B)]
            first = phase1_ops(units[0][1])
            first = (first[0:1] + [load_w] + first[1:4] + [build_w, init_vp]
                     + first[4:])
            # prefix: inits + h DMAs + quarter 0 (4 ops). The first q-slice's
            # units consume chunks in pair order; the in-loop pops (3 per
            # unit, after each unit's MMs) must emit quarter q's trq/projk/
            # vprojq (pending idx 4q-4..4q-2) before unit 2q+2's MMs (3
            # pops per earlier unit: idx < 6q+6) — ample slack for q<=3.
            npre = 11
            for op in first[:npre]:
                op()
            pending = first[npre:]
            uid = 0
            for ui, (rep, b) in enumerate(units):
                qT2, kT2, vp, ob_all = (qT_b[b], kT_b[b], vp_b[b], ob_b[b])
                if ui + 1 < len(units):
                    pending = pending + phase1_ops(units[ui + 1][1])

                vp8 = v8_b[b]
                for s in range(NSL):
                    q0 = s * QB
                    oT = popool.tile([VP1, QB], f32, tag="oT", name="oT")
                    for p in range(NPAIR):
                        c0, c1 = 2 * p, 2 * p + 1
                        cps = pcpool.tile([P, 2 * QB], f32, tag="c",
                                          name="cps")
                        nc.tensor.matmul(
                            cps[:, 0:QB],
                            kT2[0:K, c0 * P:(c0 + 1) * P],
                            qT2[0:K, q0:q0 + QB],
                            start=True, stop=True, tile_position=(0, 0))
                        nc.tensor.matmul(
                            cps[:, QB:2 * QB],
                            kT2[32:32 + K, c1 * P:(c1 + 1) * P],
                            qT2[32:32 + K, q0:q0 + QB],
                            start=True, stop=True, tile_position=(32, 0))
                        dve_unit = unit_on_dve(uid % NU)
                        if dve_unit:
                            at = apool.tile([P, 2 * QB], bf16, tag="at",
                                            name="at")
                            nc.vector.tensor_scalar(
                                at.bitcast(i16), cps, A16, B16, MULT, ADD)
                            nc.tensor.matmul(
                                oT, vp[:, c0 * VPW:c0 * VPW + VP1],
                                at[:, 0:QB],
                                start=(p == 0), stop=False,
                                skip_group_check=True)
                            nc.tensor.matmul(
                                oT, vp[:, c1 * VPW:c1 * VPW + VP1],
                                at[:, QB:2 * QB],
                                start=False, stop=(p == NPAIR - 1),
                                skip_group_check=True)
                        elif fp8_av:
                            at8 = apool.tile([P, 2 * QB], fp8, tag="at8",
                                             name="at8")
                            nc.scalar.activation(at8, cps, EXP,
                                                 scale=0.25, bias=biast)
                            nc.tensor.matmul(
                                oT,
                                vp8[:, p * 64:(p + 1) * 64].rearrange(
                                    "p (k w) -> p k w", k=2)[:, :, 0:VP1],
                                at8.rearrange("p (k n) -> p k n", k=2),
                                start=(p == 0), stop=(p == NPAIR - 1),
                                perf_mode=DR, skip_group_check=True)
                        else:
                            at = apool.tile([P, 2 * QB], bf16, tag="at",
                                            name="at")
                            nc.scalar.activation(at, cps, EXP, scale=0.25)
                            nc.tensor.matmul(
                                oT, vp[:, c0 * VPW:c0 * VPW + VP1],
                                at[:, 0:QB],
                                start=(p == 0), stop=False,
                                skip_group_check=True)
                            nc.tensor.matmul(
                                oT, vp[:, c1 * VPW:c1 * VPW + VP1],
                                at[:, QB:2 * QB],
                                start=False, stop=(p == NPAIR - 1),
                                skip_group_check=True)
                        uid += 1
                        for _ in range(cfg["pops"]):
                            if pending:
                                pending.pop(0)()

                    # normalize this q-slice: transpose the four 128-q tiles
                    # into ONE [128, 68] psum tile, then a single reciprocal
                    # + broadcast-multiply pass
                    oT_sb = apool.tile([VP1, QB], f32, tag="oTsb",
                                       name="oT_sb")
                    nc.vector.tensor_copy(oT_sb, oT)
                    pf = scpool.tile([P, QB], f32, tag="s", name="pf")
                    for tl in range(QB // P):
                        nc.tensor.transpose(
                            pf[:, tl * VP1:(tl + 1) * VP1],
                            oT_sb[:, tl * P:(tl + 1) * P],
                            ident[:VP1, :VP1])
                    pf3 = pf[:, 0:4 * VP1].rearrange("p (t w) -> p t w",
                                                     w=VP1)
                    rcp = apool.tile([P, 4], f32, tag="rcp", name="rcp")
                    nc.vector.reciprocal(rcp.unsqueeze(2),
                                         pf3[:, :, V:V + 1])
                    nc.vector.tensor_tensor(
                        ob_all[:, 4 * s * V:4 * (s + 1) * V].rearrange(
                            "p (t v) -> p t v", v=V),
                        pf3[:, :, 0:V],
                        rcp.unsqueeze(2).broadcast_to([P, 4, V]),
                        MULT)

                    # per-slice out DMA so the store overlaps the next
                    nc.sync.dma_start(
                        out_d[b, q0:q0 + QB, :].rearrange(
                            "(t p) v -> p t v", p=P),
                        ob_all[:, (q0 // P) * V:((q0 + QB) // P) * V]
                        .rearrange("p (t v) -> p t v", t=QB // P))

                for op in pending:
                    op()
                pending = []

    nc.compile()
    return nc


def _get(cfg=None):
    cfg = cfg or {}
    key = tuple(sorted({**DEFAULT_CFG, **cfg}.items()))
    if key not in _CACHE:
        _CACHE[key] = _build(key)
    return _CACHE[key]


def _in_maps(h, W_Q, W_K, W_V):
    h = np.ascontiguousarray(np.asarray(h, dtype=np.float32))
    W_Q = np.asarray(W_Q, dtype=np.float32)
    W_K = np.asarray(W_K, dtype=np.float32)
    W_V = np.asarray(W_V, dtype=np.float32)
    return [
        {"h": h, "wq": np.ascontiguousarray(W_Q[c]),
         "wk": np.ascontiguousarray(W_K[c]),
         "wv": np.ascontiguousarray(W_V[c])}
        for c in range(N_CORES)
    ]


def kernel(h, W_Q, W_K, W_V, cfg=None, **run_kwargs):
    from concourse import bass_utils
    nc = _get(cfg)
    res = bass_utils.run_bass_kernel_spmd(
        nc, _in_maps(h, W_Q, W_K, W_V),
        core_ids=list(range(N_CORES)), **run_kwargs)
    out = np.stack([res.results[c]["out"] for c in range(N_CORES)], axis=1)
    kernel.last_results = res
    return out



# revision 32
# speedup vs baseline: 2.1422x; 2.1422x over previous
"""Multi-head attention (Vaswani) on Trainium2, head-parallel across 8 NeuronCores.

Problem shapes (hardcoded):
  h:   [B=2, G=2048, D=128] f32
  W_Q/W_K/W_V: [H=8, D=128, K=16] f32
  out: [B=2, H=8, G=2048, V=16] f32  = softmax(0.25 * (h@Wq) @ (h@Wk)^T) @ (h@Wv)

Sharding: one head per core (8 heads / 8 cores). Each core receives the full h
plus its head's weight slices, computes [B, G, V]; host stacks on the head axis.

Per-core plan (v5). The kernel is fundamentally elementwise-bound: all
B*G*G = 8.4M compat logits must leave PSUM through a 1-elem/lane/cycle
engine op. v5 therefore (a) spreads that traversal over THREE engines
(ACT true exp; DVE and GPSIMD/Pool via the Schraudolph bit trick:
bits_i16 = trunc(A*0.25*qk + B) reinterpreted as bf16 ~= exp(0.25*qk);
the A factor is pre-folded into wq2 so trick units are a single
tensor_scalar add), and (b) packs the PE work with tile_position so the
matmul streams run ~4-wide concurrently:

  - 4 q-slices (512 q each) of a batch run CONCURRENTLY. Slice s
    accumulates its AV output in col-group s of ONE [128,512] PSUM tile
    (partitions 32s..32s+16), so no cross-group reduction is needed.
  - compat for slices 0/2 streams in PE row groups 0/32 and slices 1/3
    in 64/96 (kT/qT replicated 4x at partitions 0/32/64/96 by the
    projection matmuls via 4x-replicated weight columns).
  - AV matmuls (lhsT = v' [128,17], ones column accumulates the softmax
    denominator) are col-tiled 4-wide; AV for round r is emitted one
    round behind its exp so PE never parks on an unsatisfied dep.
  - normalization happens once per batch: one [128,512] PSUM->SBUF copy,
    16 tiny concurrent PE transposes (identity diag blocks 32s..32s+16),
    one strided reciprocal, one broadcast multiply, one out-DMA.

HW constraints discovered by bisection (see transcript):
  - GPSIMD (Pool) cannot touch PSUM at all -> only ACT and DVE can drain
    compat tiles; Pool gets SBUF-only work (memsets, normalize gather).
  - Two PE instructions in DIFFERENT row groups may run concurrently
    ONLY if they write DIFFERENT PSUM banks (same-bank pairs hang the
    device); col-tiled instructions may share a bank (distinct partition
    quadrants).
  - Partition-shifted tensor_copy (src/dst at different partition bases)
    works on every engine.

Measured HW rates (reps-slope micro-benches, much faster than the
TimelineSim cost model): ACT exp [128,1024] 397ns; DVE tensor_scalar
[128,1024] 477ns; compat MM (K=16,N=512, row-group-cycled) 50ns; AV MM
(M=32,N=512, col-tiled) 69ns; proj MM 117ns; f32 128x128 transpose 34ns.
A noexp diagnostic put the PE+DMA floor at ~15us/rep with exp exposing
only ~3.5us more; av_lag=2 (AV two rounds behind exp) reclaimed most of
that. The h/out DMAs use a stride-4 row permutation so h loads as 2KB
contiguous runs (not 512B) and out stores as 256B runs (not 64B): key/
value permutation cancels inside attention, and the q permutation is
undone by the out-DMA access pattern. ~19-25us/rep measured (window-
dependent) vs ~47-50us/rep for the previous (v4) kernel.

cfg: n_act = how many of the 64 (batch, round, slice) exp units run on
ACT (true exp); DVE Schraudolph takes the rest (n_pool 2-stage units
exist but are strictly worse - every unit still exits PSUM through
ACT/DVE). proj_act / hT_act: how many proj / hT psum->sbuf copies go to
ACT (rest DVE). reps chains the whole body for slope timing.
"""

import numpy as np

B, G, D = 2, 2048, 128
H, K, V = 8, 16, 16
N_CORES = 8
P = 128
GT = G // P          # 16 key chunks of 128
QB = 512             # q-slice width (one fp32 PSUM bank)
NSL = G // QB        # 4 q-slices per batch, run concurrently
NR = GT // 2         # 8 rounds (chunk pairs) per batch
VP1 = V + 1          # v' width (ones column appended)
VPW = 32             # v' chunk stride (padded to 32 so AV writes the
                     # full 32-partition col group -> no uninit PSUM)

DEFAULT_CFG = {
    "n_act": 38,       # of the 64 units, how many exp on ACT (true exp)
    "n_pool": 0,       # ... and how many 2-stage via Pool (gpsimd cannot
                       # read PSUM; 0 = plain DVE trick for the rest)
    "proj_act": 4,     # of 16 proj copies, how many on ACT (rest DVE)
    "hT_act": 4,       # of 8 hT copies, how many on ACT (rest DVE)
    "pc_bufs": 3,      # compat psum buffers ([128,1024] = 2 banks each)
    "at_bufs": 16,     # attnT sbuf buffers (>= 4*(av_lag+1) + slack)
    "av_lag": 2,       # AV matmuls emitted this many rounds behind exp
    "reps": 1,         # repeat whole kernel body (for HW slope timing)
    "pops": 2,         # staged ops popped per unit
    "noexp": False,    # DIAGNOSTIC ONLY: skip exp, AV reads a static at
                       # tile (wrong results; isolates PE+DMA floor)
}

# Schraudolph constants: exp(0.25*qk) via bf16 bit trick. A16 is folded
# into wq2 (so cps = A16*qk); B0 includes +0.5 so the f32->i16 truncation
# rounds half-up.
A16 = float(0.25 * (2 ** 7) / np.log(2.0))
B0 = float(127 * 2 ** 7) + 0.5
S_ACT = float(np.log(2.0) / (2 ** 7))   # ACT: exp(S_ACT * cps) = exp(0.25*qk)

_CACHE = {}


def _build(cfg_key):
    cfg = dict(DEFAULT_CFG)
    cfg.update(dict(cfg_key))
    import concourse.bacc as bacc
    import concourse.mybir as mybir
    from concourse.tile import TileContext
    from concourse.masks import make_identity

    f32 = mybir.dt.float32
    bf16 = mybir.dt.bfloat16
    i16 = mybir.dt.int16
    EXP = mybir.ActivationFunctionType.Exp
    MULT = mybir.AluOpType.mult
    ADD = mybir.AluOpType.add

    NU = B * NR * NSL  # 64 exp units per rep
    n_act = int(cfg["n_act"])
    n_pool = int(cfg["n_pool"])

    def unit_eng(uid):
        """Spread ACT/Pool/DVE units evenly over the unit sequence."""
        uid = uid % NU
        if (uid + 1) * n_act // NU > uid * n_act // NU:
            return "act"
        r = uid - (uid + 1) * n_act // NU  # rank among non-ACT units
        nrest = NU - n_act
        if (r + 1) * n_pool // nrest > r * n_pool // nrest:
            return "pool"
        return "dve"

    nc = bacc.Bacc("TRN2", debug=False, enable_asserts=False,
                   target_bir_lowering=False)
    h_d = nc.dram_tensor("h", [B, G, D], f32, kind="ExternalInput").ap()
    wq_d = nc.dram_tensor("wq", [D, K], f32, kind="ExternalInput").ap()
    wk_d = nc.dram_tensor("wk", [D, K], f32, kind="ExternalInput").ap()
    wv_d = nc.dram_tensor("wv", [D, V], f32, kind="ExternalInput").ap()
    out_d = nc.dram_tensor("out", [B, G, V], f32, kind="ExternalOutput").ap()

    with TileContext(nc) as tc:
        with tc.tile_pool(name="const", bufs=1) as cpool, \
             tc.tile_pool(name="sc", bufs=1, space="PSUM") as scpool, \
             tc.tile_pool(name="pc", bufs=cfg["pc_bufs"],
                          space="PSUM") as pcpool, \
             tc.tile_pool(name="po", bufs=1, space="PSUM") as popool, \
             tc.tile_pool(name="att", bufs=cfg["at_bufs"]) as apool:
            ident = cpool.tile([P, P], f32)
            make_identity(nc, ident)
            warm = cpool.tile([P, 1], f32)
            nc.scalar.activation(warm, ident[:, 0:1], EXP)
            at_static = None
            if cfg["noexp"]:
                at_static = cpool.tile([P, 2 * QB], bf16)
                nc.gpsimd.memset(at_static, 0.5)
            w_sb = cpool.tile([D, 3 * K], f32)
            wq2 = cpool.tile([D, P], bf16)
            wk2 = cpool.tile([D, P], bf16)
            wv_r = cpool.tile([D, V], bf16)

            def load_w():
                nc.sync.dma_start(w_sb[:, 0:K], wq_d)
                nc.sync.dma_start(w_sb[:, K:2 * K], wk_d)
                nc.sync.dma_start(w_sb[:, 2 * K:3 * K], wv_d)

            def build_w():
                nc.gpsimd.memset(wq2, 0.0)
                nc.gpsimd.memset(wk2, 0.0)
                # 4 replicas at cols 32g..32g+15; wq2 pre-scaled by A16
                wq4 = wq2.rearrange("d (r k) -> d r k", k=32)[:, :, 0:K]
                wk4 = wk2.rearrange("d (r k) -> d r k", k=32)[:, :, 0:K]
                src_q = w_sb[:, 0:K].unsqueeze(1).broadcast_to([D, 4, K])
                src_k = w_sb[:, K:2 * K].unsqueeze(1).broadcast_to([D, 4, K])
                nc.vector.tensor_scalar(wq4, src_q, A16, None, MULT)
                nc.vector.tensor_copy(wk4, src_k)
                nc.vector.tensor_copy(wv_r, w_sb[:, 2 * K:3 * K])

            hA_b, hT_b, qT_b, kT_b, vp_b, ob_b = [], [], [], [], [], []
            for b in range(B):
                hA_b.append(cpool.tile([P, G], f32, name=f"hA{b}"))
                hT_b.append(cpool.tile([P, G], bf16, name=f"hT{b}"))
                qT_b.append(cpool.tile([112, G], bf16, name=f"qT{b}"))
                kT_b.append(cpool.tile([112, G], bf16, name=f"kT{b}"))
                vp_b.append(cpool.tile([P, GT * VPW], bf16, name=f"vp{b}"))
                ob_b.append(cpool.tile([P, GT * V], f32, name=f"ob{b}"))

            def init_vp():
                for b in range(B):
                    nc.gpsimd.memset(vp_b[b], 0.0)
                    nc.gpsimd.memset(
                        vp_b[b].rearrange("p (t w) -> p t w", w=VPW)
                        [:, :, V:V + 1], 1.0)

            pcount = [0, 0]  # proj copy counter, hT copy counter

            def phase1_ops(b):
                """Input staging for batch b, in dependency order; popped a
                few per unit inside the previous batch's main loop."""
                hA, hT, qT2, kT2, vp = (hA_b[b], hT_b[b], qT_b[b],
                                        kT_b[b], vp_b[b])

                def dmaq(qq):
                    # partition p takes 4 CONSECUTIVE h rows (2KB contiguous
                    # DMA runs instead of 512B). hA block t then holds rows
                    # 4p+t of the quarter: keys/values are permuted the same
                    # way (softmax is key-permutation invariant) and the
                    # q-side permutation is undone by the out-DMA pattern.
                    nc.sync.dma_start(
                        hA[:, qq * 4 * P:(qq + 1) * 4 * P].rearrange(
                            "p (t d) -> p t d", t=4),
                        h_d[b, qq * 4 * P:(qq + 1) * 4 * P, :].rearrange(
                            "(p t) d -> p t d", t=4))

                def trq(qq):
                    # 4 chunk transposes into one psum bank, single copy out
                    # (f32 transpose_mode is fast on this chip: ~34ns each)
                    pt = scpool.tile([P, QB], f32, tag="s", name="pt")
                    for j in range(4):
                        t = 4 * qq + j
                        nc.tensor.transpose(pt[:, j * P:(j + 1) * P],
                                            hA[:, t * P:(t + 1) * P], ident)
                    i = pcount[1]
                    pcount[1] += 1
                    eng = nc.scalar if (i % 8) < cfg["hT_act"] else None
                    dst = hT[:, qq * 4 * P:(qq + 1) * 4 * P]
                    if eng is not None:
                        nc.scalar.copy(dst, pt)
                    else:
                        nc.vector.tensor_copy(dst, pt)

                def proj(qb, w2, dst):
                    sl = slice(qb * QB, (qb + 1) * QB)
                    pq = scpool.tile([P, QB], f32, tag="s", name="pq")
                    nc.tensor.matmul(pq, w2, hT[:, sl],
                                     start=True, stop=True)
                    i = pcount[0]
                    pcount[0] += 1
                    if (i % 16) < cfg["proj_act"]:
                        nc.scalar.copy(dst[0:112, sl], pq[0:112, :])
                    else:
                        nc.vector.tensor_copy(dst[0:112, sl], pq[0:112, :])

                def vprojq(qq):
                    # 4 chunks' v' into one psum tile, then one strided copy
                    pvv = scpool.tile([P, QB], f32, tag="s", name="pvv")
                    for j in range(4):
                        t = 4 * qq + j
                        nc.tensor.matmul(pvv[:, j * V:(j + 1) * V],
                                         hT[:, t * P:(t + 1) * P],
                                         wv_r, start=True, stop=True)
                    src = pvv[:, 0:4 * V].rearrange("p (j v) -> p j v", v=V)
                    nc.vector.tensor_copy(
                        vp.rearrange("p (t w) -> p t w", w=VPW)
                        [:, 4 * qq:4 * qq + 4, 0:V], src)

                # Round 0 consumes qT for ALL slices, so q staging comes
                # first; kT/vp quarters are consumed progressively.
                ops = [lambda qq=qq: dmaq(qq) for qq in range(4)]
                for qq in range(4):
                    ops.append(lambda qq=qq: trq(qq))
                    ops.append(lambda qq=qq: proj(qq, wq2, qT2))
                for qq in range(4):
                    ops.append(lambda qq=qq: proj(qq, wk2, kT2))
                    ops.append(lambda qq=qq: vprojq(qq))
                return ops

            units = [(rr, bb) for rr in range(cfg["reps"])
                     for bb in range(B)]
            first = phase1_ops(units[0][1])
            first = (first[0:1] + [load_w] + first[1:4] + [build_w, init_vp]
                     + first[4:])
            # prologue: everything batch 0 needs before round 0 (all of
            # qT) runs up front; k quarters 1-3 pop during early rounds.
            npre = len(first) - 6   # leave projk/vproj for qq=1..3 pending
            for op in first[:npre]:
                op()
            pending = first[npre:]
            uid = 0
            for ui, (rep, b) in enumerate(units):
                qT2, kT2, vp, ob_all = (qT_b[b], kT_b[b], vp_b[b], ob_b[b])
                if ui + 1 < len(units):
                    pending = pending + phase1_ops(units[ui + 1][1])

                oT4 = popool.tile([P, QB], f32, tag="oT", name="oT4")
                # AV for round r is emitted av_lag rounds later so the PE
                # never parks on an unsatisfied exp dependency
                av_q = []
                LAG = int(cfg["av_lag"])

                def emit_av(avs, r):
                    # half-outer order: the 4 col groups' streams start
                    # back-to-back (disjoint col strips run concurrently);
                    # same-group chunk pairs serialize behind them.
                    for half in range(2):
                        c = 2 * r + half
                        for s in range(NSL):
                            nc.tensor.matmul(
                                oT4[32 * s:32 * s + 32, :],
                                vp[:, c * VPW:c * VPW + 32],
                                avs[s][:, half * QB:(half + 1) * QB],
                                start=(r == 0 and half == 0),
                                stop=(r == NR - 1 and half == 1),
                                skip_group_check=True,
                                tile_position=(0, 32 * s))

                for r in range(NR):
                    c0, c1 = 2 * r, 2 * r + 1
                    avs = []
                    for s in range(NSL):
                        q0 = s * QB
                        cps = pcpool.tile([P, 2 * QB], f32, tag="c",
                                          name="cps")
                        rg = 64 * (s % 2)  # row groups 0/32 or 64/96
                        nc.tensor.matmul(
                            cps[:, 0:QB],
                            kT2[rg:rg + K, c0 * P:(c0 + 1) * P],
                            qT2[rg:rg + K, q0:q0 + QB],
                            start=True, stop=True, tile_position=(rg, 0))
                        nc.tensor.matmul(
                            cps[:, QB:2 * QB],
                            kT2[rg + 32:rg + 32 + K, c1 * P:(c1 + 1) * P],
                            qT2[rg + 32:rg + 32 + K, q0:q0 + QB],
                            start=True, stop=True,
                            tile_position=(rg + 32, 0))
                        if cfg["noexp"]:
                            avs.append(at_static)
                            uid += 1
                            for _ in range(cfg["pops"]):
                                if pending:
                                    pending.pop(0)()
                            continue
                        at = apool.tile([P, 2 * QB], bf16, tag="at",
                                        name="at")
                        eng = unit_eng(uid)
                        if eng == "act":
                            nc.scalar.activation(at, cps, EXP, scale=S_ACT)
                        elif eng == "pool":
                            # 2-stage: DVE drains PSUM->SBUF f32, Pool
                            # (which cannot read PSUM) does the trick
                            # SBUF->SBUF
                            cp_sb = apool.tile([P, 2 * QB], f32, tag="cp",
                                               name="cp_sb", bufs=3)
                            nc.vector.tensor_copy(cp_sb, cps)
                            nc.gpsimd.tensor_scalar(
                                at.bitcast(i16), cp_sb, B0, None, ADD)
                        else:
                            nc.vector.tensor_scalar(
                                at.bitcast(i16), cps, B0, None, ADD)
                        avs.append(at)
                        uid += 1
                        for _ in range(cfg["pops"]):
                            if pending:
                                pending.pop(0)()
                    av_q.append(avs)
                    if len(av_q) > LAG:
                        emit_av(av_q.pop(0), r - LAG)
                for i, avs in enumerate(av_q):
                    emit_av(avs, NR - len(av_q) + i)

                # normalize the whole batch: one PSUM->SBUF copy (ACT),
                # then gather groups 1-3 to partition base 0 with Pool
                # partition-shifted SBUF copies (concurrent PE tiles must
                # not share a PSUM bank, so the 16 transposes all run in
                # row group 0, serially, into ONE reused po-tag bank),
                # one reciprocal, one broadcast multiply, one out-DMA
                oT_sb = apool.tile([P, QB], f32, tag="oTsb", name="oT_sb",
                                   bufs=2)
                nc.scalar.copy(oT_sb, oT4)
                oT_all = apool.tile([VP1, 3 * QB], f32, tag="oTall",
                                    name="oT_all", bufs=2)
                # one gather copy per engine so the critical path into the
                # transposes is a single 512-wide copy (DVE is 2x on
                # all-SBUF f32)
                for s, geng in ((1, nc.vector), (2, nc.gpsimd),
                                (3, nc.scalar)):
                    if geng is nc.scalar:
                        nc.scalar.copy(oT_all[0:VP1, (s - 1) * QB:s * QB],
                                       oT_sb[32 * s:32 * s + VP1, :])
                    else:
                        geng.tensor_copy(
                            oT_all[0:VP1, (s - 1) * QB:s * QB],
                            oT_sb[32 * s:32 * s + VP1, :])
                pf = popool.tile([P, QB], f32, tag="oT", name="pf")
                for s in range(NSL):
                    for tl in range(QB // P):
                        u = s * 4 + tl
                        src = (oT_sb[0:VP1, tl * P:(tl + 1) * P] if s == 0
                               else oT_all[0:VP1,
                                           (s - 1) * QB + tl * P:
                                           (s - 1) * QB + (tl + 1) * P])
                        nc.tensor.transpose(
                            pf[:, u * VP1:(u + 1) * VP1],
                            src, ident[0:VP1, 0:VP1])
                pf3 = pf[:, 0:GT * VP1].rearrange("p (t w) -> p t w", w=VP1)
                rcp = apool.tile([P, GT], f32, tag="rcp", name="rcp")
                nc.vector.reciprocal(rcp.unsqueeze(2), pf3[:, :, V:V + 1])
                nc.vector.tensor_tensor(
                    ob_all.rearrange("p (t v) -> p t v", v=V),
                    pf3[:, :, 0:V],
                    rcp.unsqueeze(2).broadcast_to([P, GT, V]),
                    MULT)
                # slot (s, tl) partition p holds q-row s*512 + 4p + tl, so
                # for fixed (p, s) the tl=0..3 rows are consecutive in DRAM:
                # 256B contiguous runs (4x fewer descriptors than row-wise)
                for s in range(NSL):
                    nc.sync.dma_start(
                        out_d[b, s * QB:(s + 1) * QB, :].rearrange(
                            "(p t) v -> p t v", t=4),
                        ob_all[:, 4 * s * V:4 * (s + 1) * V].rearrange(
                            "p (t v) -> p t v", v=V))

                for op in pending:
                    op()
                pending = []

    nc.compile()
    return nc


def _get(cfg=None):
    cfg = cfg or {}
    key = tuple(sorted({**DEFAULT_CFG, **cfg}.items()))
    if key not in _CACHE:
        _CACHE[key] = _build(key)
    return _CACHE[key]


def _in_maps(h, W_Q, W_K, W_V):
    h = np.ascontiguousarray(np.asarray(h, dtype=np.float32))
    W_Q = np.asarray(W_Q, dtype=np.float32)
    W_K = np.asarray(W_K, dtype=np.float32)
    W_V = np.asarray(W_V, dtype=np.float32)
    return [
        {"h": h, "wq": np.ascontiguousarray(W_Q[c]),
         "wk": np.ascontiguousarray(W_K[c]),
         "wv": np.ascontiguousarray(W_V[c])}
        for c in range(N_CORES)
    ]


def kernel(h, W_Q, W_K, W_V, cfg=None, **run_kwargs):
    from concourse import bass_utils
    nc = _get(cfg)
    res = bass_utils.run_bass_kernel_spmd(
        nc, _in_maps(h, W_Q, W_K, W_V),
        core_ids=list(range(N_CORES)), **run_kwargs)
    out = np.stack([res.results[c]["out"] for c in range(N_CORES)], axis=1)
    kernel.last_results = res
    return out


# revision 34
# speedup vs baseline: 3.3867x; 1.5809x over previous
"""Multi-head attention (Vaswani) on Trainium2, head-parallel across 8 NeuronCores.

Problem shapes (hardcoded):
  h:   [B=2, G=2048, D=128] f32
  W_Q/W_K/W_V: [H=8, D=128, K=16] f32
  out: [B=2, H=8, G=2048, V=16] f32  = softmax(0.25 * (h@Wq) @ (h@Wk)^T) @ (h@Wv)

Sharding: one head per core (8 heads / 8 cores). Each core receives the full h
plus its head's weight slices, computes [B, G, V]; host stacks on the head axis.

Per-core plan (v5). The kernel is fundamentally elementwise-bound: all
B*G*G = 8.4M compat logits must leave PSUM through a 1-elem/lane/cycle
engine op. v5 therefore (a) spreads that traversal over THREE engines
(ACT true exp; DVE and GPSIMD/Pool via the Schraudolph bit trick:
bits_i16 = trunc(A*0.25*qk + B) reinterpreted as bf16 ~= exp(0.25*qk);
the A factor is pre-folded into wq2 so trick units are a single
tensor_scalar add), and (b) packs the PE work with tile_position so the
matmul streams run ~4-wide concurrently:

  - 4 q-slices (512 q each) of a batch run CONCURRENTLY. Slice s
    accumulates its AV output in col-group s of ONE [128,512] PSUM tile
    (partitions 32s..32s+16), so no cross-group reduction is needed.
  - compat for slices 0/2 streams in PE row groups 0/32 and slices 1/3
    in 64/96 (kT/qT replicated 4x at partitions 0/32/64/96 by the
    projection matmuls via 4x-replicated weight columns).
  - AV matmuls (lhsT = v' [128,17], ones column accumulates the softmax
    denominator) are col-tiled 4-wide; AV for round r is emitted one
    round behind its exp so PE never parks on an unsatisfied dep.
  - normalization happens once per batch: one [128,512] PSUM->SBUF copy,
    16 tiny concurrent PE transposes (identity diag blocks 32s..32s+16),
    one strided reciprocal, one broadcast multiply, one out-DMA.

HW constraints discovered by bisection (see transcript):
  - GPSIMD (Pool) cannot touch PSUM at all -> only ACT and DVE can drain
    compat tiles; Pool gets SBUF-only work (memsets, normalize gather).
  - Two PE instructions in DIFFERENT row groups may run concurrently
    ONLY if they write DIFFERENT PSUM banks (same-bank pairs hang the
    device); col-tiled instructions may share a bank (distinct partition
    quadrants).
  - Partition-shifted tensor_copy (src/dst at different partition bases)
    works on every engine.

Measured HW rates (reps-slope micro-benches, much faster than the
TimelineSim cost model): ACT exp [128,1024] 397ns; DVE tensor_scalar
[128,1024] 477ns; compat MM (K=16,N=512, row-group-cycled) 50ns; AV MM
(M=32,N=512, col-tiled) 69ns; proj MM 117ns; f32 128x128 transpose 34ns.
A noexp diagnostic put the PE+DMA floor at ~15us/rep with exp exposing
only ~3.5us more; av_lag=2 (AV two rounds behind exp) reclaimed most of
that. The h/out DMAs use a stride-4 row permutation so h loads as 2KB
contiguous runs (not 512B) and out stores as 256B runs (not 64B): key/
value permutation cancels inside attention, and the q permutation is
undone by the out-DMA access pattern. ~19-25us/rep measured (window-
dependent) vs ~47-50us/rep for the previous (v4) kernel.

cfg: n_act = how many of the 64 (batch, round, slice) exp units run on
ACT (true exp); DVE Schraudolph takes the rest (n_pool 2-stage units
exist but are strictly worse - every unit still exits PSUM through
ACT/DVE). proj_act / hT_act: how many proj / hT psum->sbuf copies go to
ACT (rest DVE). reps chains the whole body for slope timing.
"""

import numpy as np

B, G, D = 2, 2048, 128
H, K, V = 8, 16, 16
N_CORES = 8
P = 128
GT = G // P          # 16 key chunks of 128
QB = 512             # q-slice width (one fp32 PSUM bank)
NSL = G // QB        # 4 q-slices per batch, run concurrently
NR = GT // 2         # 8 rounds (chunk pairs) per batch
VP1 = V + 1          # v' width (ones column appended)
VPW = 32             # v' chunk stride (padded to 32 so AV writes the
                     # full 32-partition col group -> no uninit PSUM)

DEFAULT_CFG = {
    "n_act": 38,       # of the 64 units, how many exp on ACT (true exp)
    "n_pool": 0,       # ... and how many 2-stage via Pool (gpsimd cannot
                       # read PSUM; 0 = plain DVE trick for the rest)
    "proj_act": 4,     # of 16 proj copies, how many on ACT (rest DVE)
    "hT_act": 4,       # of 8 hT copies, how many on ACT (rest DVE)
    "pc_bufs": 3,      # compat psum buffers ([128,1024] = 2 banks each)
    "at_bufs": 16,     # attnT sbuf buffers (>= 4*(av_lag+1) + slack)
    "av_lag": 2,       # AV matmuls emitted this many rounds behind exp
    "reps": 1,         # repeat whole kernel body (for HW slope timing)
    "pops": 23,        # staged ops popped per unit. 23 = drain the whole
                       # next-batch staging list at the first unit: beat
                       # pops=3 by ~10-15us/rep (the drip-feed serialized
                       # the staging chain across the batch)
    "noexp": False,    # DIAGNOSTIC ONLY: skip exp, AV reads a static at
                       # tile (wrong results; isolates PE+DMA floor)
}

# Schraudolph constants: exp(0.25*qk) via bf16 bit trick. A16 is folded
# into wq2 (so cps = A16*qk); B0 includes +0.5 so the f32->i16 truncation
# rounds half-up.
A16 = float(0.25 * (2 ** 7) / np.log(2.0))
B0 = float(127 * 2 ** 7) + 0.5
S_ACT = float(np.log(2.0) / (2 ** 7))   # ACT: exp(S_ACT * cps) = exp(0.25*qk)

_CACHE = {}


def _build(cfg_key):
    cfg = dict(DEFAULT_CFG)
    cfg.update(dict(cfg_key))
    import concourse.bacc as bacc
    import concourse.mybir as mybir
    from concourse.tile import TileContext
    from concourse.masks import make_identity

    f32 = mybir.dt.float32
    bf16 = mybir.dt.bfloat16
    i16 = mybir.dt.int16
    EXP = mybir.ActivationFunctionType.Exp
    MULT = mybir.AluOpType.mult
    ADD = mybir.AluOpType.add

    NU = B * NR * NSL  # 64 exp units per rep
    n_act = int(cfg["n_act"])
    n_pool = int(cfg["n_pool"])

    def unit_eng(uid):
        """Spread ACT/Pool/DVE units evenly over the unit sequence."""
        uid = uid % NU
        if (uid + 1) * n_act // NU > uid * n_act // NU:
            return "act"
        r = uid - (uid + 1) * n_act // NU  # rank among non-ACT units
        nrest = NU - n_act
        if (r + 1) * n_pool // nrest > r * n_pool // nrest:
            return "pool"
        return "dve"

    nc = bacc.Bacc("TRN2", debug=False, enable_asserts=False,
                   target_bir_lowering=False)
    h_d = nc.dram_tensor("h", [B, G, D], f32, kind="ExternalInput").ap()
    wq_d = nc.dram_tensor("wq", [D, K], f32, kind="ExternalInput").ap()
    wk_d = nc.dram_tensor("wk", [D, K], f32, kind="ExternalInput").ap()
    wv_d = nc.dram_tensor("wv", [D, V], f32, kind="ExternalInput").ap()
    out_d = nc.dram_tensor("out", [B, G, V], f32, kind="ExternalOutput").ap()

    with TileContext(nc) as tc:
        with tc.tile_pool(name="const", bufs=1) as cpool, \
             tc.tile_pool(name="sc", bufs=1, space="PSUM") as scpool, \
             tc.tile_pool(name="pc", bufs=cfg["pc_bufs"],
                          space="PSUM") as pcpool, \
             tc.tile_pool(name="po", bufs=1, space="PSUM") as popool, \
             tc.tile_pool(name="att", bufs=cfg["at_bufs"]) as apool:
            ident = cpool.tile([P, P], f32)
            make_identity(nc, ident)
            warm = cpool.tile([P, 1], f32)
            nc.scalar.activation(warm, ident[:, 0:1], EXP)
            at_static = None
            if cfg["noexp"]:
                at_static = cpool.tile([P, 2 * QB], bf16)
                nc.gpsimd.memset(at_static, 0.5)
            w_sb = cpool.tile([D, 3 * K], f32)
            wq2 = cpool.tile([D, P], bf16)
            wk2 = cpool.tile([D, P], bf16)
            wv_r = cpool.tile([D, V], bf16)

            def load_w():
                nc.sync.dma_start(w_sb[:, 0:K], wq_d)
                nc.sync.dma_start(w_sb[:, K:2 * K], wk_d)
                nc.sync.dma_start(w_sb[:, 2 * K:3 * K], wv_d)

            def build_w():
                nc.gpsimd.memset(wq2, 0.0)
                nc.gpsimd.memset(wk2, 0.0)
                # 4 replicas at cols 32g..32g+15; wq2 pre-scaled by A16
                wq4 = wq2.rearrange("d (r k) -> d r k", k=32)[:, :, 0:K]
                wk4 = wk2.rearrange("d (r k) -> d r k", k=32)[:, :, 0:K]
                src_q = w_sb[:, 0:K].unsqueeze(1).broadcast_to([D, 4, K])
                src_k = w_sb[:, K:2 * K].unsqueeze(1).broadcast_to([D, 4, K])
                nc.vector.tensor_scalar(wq4, src_q, A16, None, MULT)
                nc.vector.tensor_copy(wk4, src_k)
                nc.vector.tensor_copy(wv_r, w_sb[:, 2 * K:3 * K])

            hA_b, hT_b, qT_b, kT_b, vp_b, ob_b = [], [], [], [], [], []
            for b in range(B):
                hA_b.append(cpool.tile([P, G], f32, name=f"hA{b}"))
                hT_b.append(cpool.tile([P, G], bf16, name=f"hT{b}"))
                qT_b.append(cpool.tile([112, G], bf16, name=f"qT{b}"))
                kT_b.append(cpool.tile([112, G], bf16, name=f"kT{b}"))
                vp_b.append(cpool.tile([P, GT * VPW], bf16, name=f"vp{b}"))
                ob_b.append(cpool.tile([P, GT * V], f32, name=f"ob{b}"))

            def init_vp():
                for b in range(B):
                    nc.gpsimd.memset(vp_b[b], 0.0)
                    nc.gpsimd.memset(
                        vp_b[b].rearrange("p (t w) -> p t w", w=VPW)
                        [:, :, V:V + 1], 1.0)

            pcount = [0, 0]  # proj copy counter, hT copy counter

            def phase1_ops(b):
                """Input staging for batch b, in dependency order; popped a
                few per unit inside the previous batch's main loop."""
                hA, hT, qT2, kT2, vp = (hA_b[b], hT_b[b], qT_b[b],
                                        kT_b[b], vp_b[b])

                def dmaq(qq):
                    # partition p takes 4 CONSECUTIVE h rows (2KB contiguous
                    # DMA runs instead of 512B). hA block t then holds rows
                    # 4p+t of the quarter: keys/values are permuted the same
                    # way (softmax is key-permutation invariant) and the
                    # q-side permutation is undone by the out-DMA pattern.
                    nc.sync.dma_start(
                        hA[:, qq * 4 * P:(qq + 1) * 4 * P].rearrange(
                            "p (t d) -> p t d", t=4),
                        h_d[b, qq * 4 * P:(qq + 1) * 4 * P, :].rearrange(
                            "(p t) d -> p t d", t=4))

                def trq(qq):
                    # 4 chunk transposes into one psum bank, single copy out
                    # (f32 transpose_mode is fast on this chip: ~34ns each)
                    pt = scpool.tile([P, QB], f32, tag="s", name="pt")
                    for j in range(4):
                        t = 4 * qq + j
                        nc.tensor.transpose(pt[:, j * P:(j + 1) * P],
                                            hA[:, t * P:(t + 1) * P], ident)
                    i = pcount[1]
                    pcount[1] += 1
                    eng = nc.scalar if (i % 8) < cfg["hT_act"] else None
                    dst = hT[:, qq * 4 * P:(qq + 1) * 4 * P]
                    if eng is not None:
                        nc.scalar.copy(dst, pt)
                    else:
                        nc.vector.tensor_copy(dst, pt)

                def proj(qb, w2, dst):
                    sl = slice(qb * QB, (qb + 1) * QB)
                    pq = scpool.tile([P, QB], f32, tag="s", name="pq")
                    nc.tensor.matmul(pq, w2, hT[:, sl],
                                     start=True, stop=True)
                    i = pcount[0]
                    pcount[0] += 1
                    if (i % 16) < cfg["proj_act"]:
                        nc.scalar.copy(dst[0:112, sl], pq[0:112, :])
                    else:
                        nc.vector.tensor_copy(dst[0:112, sl], pq[0:112, :])

                def vprojq(qq):
                    # 4 chunks' v' into one psum tile, then one strided copy
                    pvv = scpool.tile([P, QB], f32, tag="s", name="pvv")
                    for j in range(4):
                        t = 4 * qq + j
                        nc.tensor.matmul(pvv[:, j * V:(j + 1) * V],
                                         hT[:, t * P:(t + 1) * P],
                                         wv_r, start=True, stop=True)
                    src = pvv[:, 0:4 * V].rearrange("p (j v) -> p j v", v=V)
                    nc.vector.tensor_copy(
                        vp.rearrange("p (t w) -> p t w", w=VPW)
                        [:, 4 * qq:4 * qq + 4, 0:V], src)

                # Round 0 consumes qT for ALL slices, so q staging comes
                # first; kT/vp quarters are consumed progressively.
                ops = [lambda qq=qq: dmaq(qq) for qq in range(4)]
                for qq in range(4):
                    ops.append(lambda qq=qq: trq(qq))
                    ops.append(lambda qq=qq: proj(qq, wq2, qT2))
                for qq in range(4):
                    ops.append(lambda qq=qq: proj(qq, wk2, kT2))
                    ops.append(lambda qq=qq: vprojq(qq))
                return ops

            units = [(rr, bb) for rr in range(cfg["reps"])
                     for bb in range(B)]
            first = phase1_ops(units[0][1])
            first = (first[0:1] + [load_w] + first[1:4] + [build_w, init_vp]
                     + first[4:])
            # prologue: everything batch 0 needs before round 0 (all of
            # qT) runs up front; k quarters 1-3 pop during early rounds.
            npre = len(first) - 6   # leave projk/vproj for qq=1..3 pending
            for op in first[:npre]:
                op()
            pending = first[npre:]
            uid = 0
            for ui, (rep, b) in enumerate(units):
                qT2, kT2, vp, ob_all = (qT_b[b], kT_b[b], vp_b[b], ob_b[b])
                if ui + 1 < len(units):
                    pending = pending + phase1_ops(units[ui + 1][1])

                oT4 = popool.tile([P, QB], f32, tag="oT", name="oT4")
                # AV for round r is emitted av_lag rounds later so the PE
                # never parks on an unsatisfied exp dependency
                av_q = []
                LAG = int(cfg["av_lag"])

                def emit_av(avs, r):
                    # half-outer order: the 4 col groups' streams start
                    # back-to-back (disjoint col strips run concurrently);
                    # same-group chunk pairs serialize behind them.
                    for half in range(2):
                        c = 2 * r + half
                        for s in range(NSL):
                            nc.tensor.matmul(
                                oT4[32 * s:32 * s + 32, :],
                                vp[:, c * VPW:c * VPW + 32],
                                avs[s][:, half * QB:(half + 1) * QB],
                                start=(r == 0 and half == 0),
                                stop=(r == NR - 1 and half == 1),
                                skip_group_check=True,
                                tile_position=(0, 32 * s))

                for r in range(NR):
                    c0, c1 = 2 * r, 2 * r + 1
                    avs = []
                    for s in range(NSL):
                        q0 = s * QB
                        cps = pcpool.tile([P, 2 * QB], f32, tag="c",
                                          name="cps")
                        rg = 64 * (s % 2)  # row groups 0/32 or 64/96
                        nc.tensor.matmul(
                            cps[:, 0:QB],
                            kT2[rg:rg + K, c0 * P:(c0 + 1) * P],
                            qT2[rg:rg + K, q0:q0 + QB],
                            start=True, stop=True, tile_position=(rg, 0))
                        nc.tensor.matmul(
                            cps[:, QB:2 * QB],
                            kT2[rg + 32:rg + 32 + K, c1 * P:(c1 + 1) * P],
                            qT2[rg + 32:rg + 32 + K, q0:q0 + QB],
                            start=True, stop=True,
                            tile_position=(rg + 32, 0))
                        if cfg["noexp"]:
                            avs.append(at_static)
                            uid += 1
                            for _ in range(cfg["pops"]):
                                if pending:
                                    pending.pop(0)()
                            continue
                        at = apool.tile([P, 2 * QB], bf16, tag="at",
                                        name="at")
                        eng = unit_eng(uid)
                        if eng == "act":
                            nc.scalar.activation(at, cps, EXP, scale=S_ACT)
                        elif eng == "pool":
                            # 2-stage: DVE drains PSUM->SBUF f32, Pool
                            # (which cannot read PSUM) does the trick
                            # SBUF->SBUF
                            cp_sb = apool.tile([P, 2 * QB], f32, tag="cp",
                                               name="cp_sb", bufs=3)
                            nc.vector.tensor_copy(cp_sb, cps)
                            nc.gpsimd.tensor_scalar(
                                at.bitcast(i16), cp_sb, B0, None, ADD)
                        else:
                            nc.vector.tensor_scalar(
                                at.bitcast(i16), cps, B0, None, ADD)
                        avs.append(at)
                        uid += 1
                        for _ in range(cfg["pops"]):
                            if pending:
                                pending.pop(0)()
                    av_q.append(avs)
                    if len(av_q) > LAG:
                        emit_av(av_q.pop(0), r - LAG)
                for i, avs in enumerate(av_q):
                    emit_av(avs, NR - len(av_q) + i)

                # normalize the whole batch: one PSUM->SBUF copy (ACT),
                # then gather groups 1-3 to partition base 0 with Pool
                # partition-shifted SBUF copies (concurrent PE tiles must
                # not share a PSUM bank, so the 16 transposes all run in
                # row group 0, serially, into ONE reused po-tag bank),
                # one reciprocal, one broadcast multiply, one out-DMA
                oT_sb = apool.tile([P, QB], f32, tag="oTsb", name="oT_sb",
                                   bufs=2)
                nc.scalar.copy(oT_sb, oT4)
                oT_all = apool.tile([VP1, 3 * QB], f32, tag="oTall",
                                    name="oT_all", bufs=2)
                # one gather copy per engine so the critical path into the
                # transposes is a single 512-wide copy (DVE is 2x on
                # all-SBUF f32)
                for s, geng in ((1, nc.vector), (2, nc.gpsimd),
                                (3, nc.scalar)):
                    if geng is nc.scalar:
                        nc.scalar.copy(oT_all[0:VP1, (s - 1) * QB:s * QB],
                                       oT_sb[32 * s:32 * s + VP1, :])
                    else:
                        geng.tensor_copy(
                            oT_all[0:VP1, (s - 1) * QB:s * QB],
                            oT_sb[32 * s:32 * s + VP1, :])
                pf = popool.tile([P, QB], f32, tag="oT", name="pf")
                for s in range(NSL):
                    for tl in range(QB // P):
                        u = s * 4 + tl
                        src = (oT_sb[0:VP1, tl * P:(tl + 1) * P] if s == 0
                               else oT_all[0:VP1,
                                           (s - 1) * QB + tl * P:
                                           (s - 1) * QB + (tl + 1) * P])
                        nc.tensor.transpose(
                            pf[:, u * VP1:(u + 1) * VP1],
                            src, ident[0:VP1, 0:VP1])
                pf3 = pf[:, 0:GT * VP1].rearrange("p (t w) -> p t w", w=VP1)
                rcp = apool.tile([P, GT], f32, tag="rcp", name="rcp")
                nc.vector.reciprocal(rcp.unsqueeze(2), pf3[:, :, V:V + 1])
                nc.vector.tensor_tensor(
                    ob_all.rearrange("p (t v) -> p t v", v=V),
                    pf3[:, :, 0:V],
                    rcp.unsqueeze(2).broadcast_to([P, GT, V]),
                    MULT)
                # slot (s, tl) partition p holds q-row s*512 + 4p + tl, so
                # for fixed (p, s) the tl=0..3 rows are consecutive in DRAM:
                # 256B contiguous runs (4x fewer descriptors than row-wise)
                for s in range(NSL):
                    nc.sync.dma_start(
                        out_d[b, s * QB:(s + 1) * QB, :].rearrange(
                            "(p t) v -> p t v", t=4),
                        ob_all[:, 4 * s * V:4 * (s + 1) * V].rearrange(
                            "p (t v) -> p t v", v=V))

                for op in pending:
                    op()
                pending = []

    nc.compile()
    return nc


def _get(cfg=None):
    cfg = cfg or {}
    key = tuple(sorted({**DEFAULT_CFG, **cfg}.items()))
    if key not in _CACHE:
        _CACHE[key] = _build(key)
    return _CACHE[key]


def _in_maps(h, W_Q, W_K, W_V):
    h = np.ascontiguousarray(np.asarray(h, dtype=np.float32))
    W_Q = np.asarray(W_Q, dtype=np.float32)
    W_K = np.asarray(W_K, dtype=np.float32)
    W_V = np.asarray(W_V, dtype=np.float32)
    return [
        {"h": h, "wq": np.ascontiguousarray(W_Q[c]),
         "wk": np.ascontiguousarray(W_K[c]),
         "wv": np.ascontiguousarray(W_V[c])}
        for c in range(N_CORES)
    ]


def kernel(h, W_Q, W_K, W_V, cfg=None, **run_kwargs):
    from concourse import bass_utils
    nc = _get(cfg)
    res = bass_utils.run_bass_kernel_spmd(
        nc, _in_maps(h, W_Q, W_K, W_V),
        core_ids=list(range(N_CORES)), **run_kwargs)
    out = np.stack([res.results[c]["out"] for c in range(N_CORES)], axis=1)
    kernel.last_results = res
    return out
